# revision 15
# baseline (speedup 1.0000x reference)
"""Trainium2 Bass kernel for L2Prompt retrieval-knn.

Reference semantics (B=4096, L=1250, P=2048, k=5):
    x = ppg[:, 0, :]                               # [B, L]
    cos[b, p]  = <x_b, key_p> / max(|x_b||key_p|, 1e-8)
    score_     = 1 - cos                           # [B, P]
    score, idx = top-k smallest score_ per row
    entropy    = mean_b H(softmax(score_[b]))
    prompted   = ppg + mean_j prompt[idx[b, j]]
    returns (prompted [B,1,L], score.mean(), entropy)

Strategy: data-parallel over B across 8 cores (512 rows each); keys/prompt
replicated. Host pre-normalizes x and keys (so the device matmul produces
cos directly) and pre-transposes them into [L, *] layout for the PE.
Per core: fp32r matmuls -> ns = cos - 1 -> vector.max (top-8) + max_index,
softmax entropy via Exp(accum) + tensor_tensor_reduce, indirect-DMA gather
of the k selected prompt rows, average, add ppg. Scalar means reduced on
host (8 tiny arrays).
"""

from contextlib import ExitStack

import numpy as np

import concourse.bacc as bacc
import concourse.bass as bass
import concourse.mybir as mybir
import concourse.tile as tile
from concourse.bass_utils import run_bass_kernel_spmd

FP32 = mybir.dt.float32
FP32R = mybir.dt.float32r

USE_FP32R = False


def _mm(ap):
    return ap.bitcast(FP32R) if USE_FP32R else ap

N_CORES = 8
B_FULL, L, P = 4096, 1250, 2048
BS = B_FULL // N_CORES  # 512 rows per core
NCH = BS // 128  # 4 chunks of 128 rows
NP = P // 512  # 4 psum column blocks
KT = [(j * 128, min(128, L - j * 128)) for j in range((L + 127) // 128)]
EPS = 1e-8

_NC_CACHE: dict[int, bass.Bass] = {}
_LAST_IN_MAPS: list | None = None


def _build(k: int) -> bass.Bass:
    # Bacc (not plain Bass): its compile() runs generate_event_semaphores,
    # which splits multi-semaphore waits into InstEventSemaphore pairs —
    # TRN2 allows at most one wait per regular instruction.
    nc = bacc.Bacc()
    # kx = [keysT | xT] fused so each K-tile arrives in ONE DMA (keeps the
    # matmul's semaphore wait count within the ISA limit).
    kx = nc.declare_dram_parameter("kx", [L, P + BS], FP32, isOutput=False)
    ppg = nc.declare_dram_parameter("ppg", [BS, L], FP32, isOutput=False)
    prompt = nc.declare_dram_parameter("prompt", [P, L], FP32, isOutput=False)
    out = nc.declare_dram_parameter("out", [BS, L], FP32, isOutput=True)
    tvals = nc.declare_dram_parameter("tvals", [BS, 8], FP32, isOutput=True)
    ent = nc.declare_dram_parameter("ent", [BS, 1], FP32, isOutput=True)

    with tile.TileContext(nc) as tc, ExitStack() as ctx:
        const = ctx.enter_context(tc.tile_pool(name="const", bufs=1))
        work = ctx.enter_context(tc.tile_pool(name="work", bufs=2))
        psum = ctx.enter_context(tc.tile_pool(name="psum", bufs=2, space="PSUM"))

        # Matmul operands, resident for the whole kernel.
        kxs = []
        for j, (off, sz) in enumerate(KT):
            kxt = const.tile([128, P + BS], FP32, tag=f"kx{j}", name=f"kx{j}")
            nc.sync.dma_start(out=kxt[:sz, :], in_=kx[off : off + sz, :])
            kxs.append(kxt)

        for i in range(NCH):
            r0 = i * 128
            ppg_t = work.tile([128, L], FP32, tag="ppg", name=f"ppg{i}")
            nc.sync.dma_start(out=ppg_t[:], in_=ppg[r0 : r0 + 128, :])

            # ns = cos - 1  (so the k smallest scores are the 8 largest ns)
            ns = work.tile([128, P], FP32, tag="ns", name=f"ns{i}")
            for n in range(NP):
                ps = psum.tile([128, 512], FP32, tag=f"ps{n}", name=f"ps{n}_{i}")
                for j, (off, sz) in enumerate(KT):
                    nc.tensor.matmul(
                        out=ps[:, :],
                        lhsT=_mm(kxs[j][:sz, P + r0 : P + r0 + 128]),
                        rhs=_mm(kxs[j][:sz, n * 512 : (n + 1) * 512]),
                        start=(j == 0),
                        stop=(j == len(KT) - 1),
                    )
                nc.scalar.activation(
                    ns[:, n * 512 : (n + 1) * 512],
                    ps[:, :],
                    mybir.ActivationFunctionType.Copy,
                    bias=-1.0,
                    scale=1.0,
                )

            vals = work.tile([128, 8], FP32, tag="vals", name=f"vals{i}")
            idx = work.tile([128, 8], mybir.dt.uint32, tag="idx", name=f"idx{i}")
            nc.vector.max(out=vals[:], in_=ns[:])
            nc.vector.max_index(out=idx[:], in_max=vals[:], in_values=ns[:])
            nc.gpsimd.dma_start(out=tvals[r0 : r0 + 128, :], in_=vals[:])

            # softmax(score_) entropy: p = e^{s}/Z with s = -ns (shift-free:
            # cos is bounded so exp(s) cannot overflow).
            # H = lnZ - sum(p*s) = lnZ + sum(e*ns)/Z
            e = work.tile([128, P], FP32, tag="e", name=f"e{i}")
            Z = work.tile([128, 1], FP32, tag="Z", name=f"Z{i}")
            nc.scalar.activation(
                e[:], ns[:], mybir.ActivationFunctionType.Exp, scale=-1.0, accum_out=Z[:]
            )
            S = work.tile([128, 1], FP32, tag="S", name=f"S{i}")
            nc.vector.tensor_tensor(
                out=e[:], in0=e[:], in1=ns[:], op=mybir.AluOpType.mult
            )
            nc.vector.reduce_sum(out=S[:], in_=e[:], axis=mybir.AxisListType.X)
            rZ = work.tile([128, 1], FP32, tag="rZ", name=f"rZ{i}")
            nc.vector.reciprocal(rZ[:], Z[:])
            lnZ = work.tile([128, 1], FP32, tag="lnZ", name=f"lnZ{i}")
            nc.scalar.activation(lnZ[:], Z[:], mybir.ActivationFunctionType.Ln)
            H = work.tile([128, 1], FP32, tag="H", name=f"H{i}")
            nc.vector.tensor_tensor(out=H[:], in0=S[:], in1=rZ[:], op=mybir.AluOpType.mult)
            nc.vector.tensor_add(H[:], H[:], lnZ[:])
            nc.gpsimd.dma_start(out=ent[r0 : r0 + 128, :], in_=H[:])

            # Gather the k selected prompt rows per output row and average.
            acc = work.tile([128, L], FP32, tag="acc", name=f"acc{i}")
            prev = None
            for j in range(k):
                g = work.tile([128, L], FP32, tag="g", name=f"g{i}_{j}", bufs=3)
                nc.gpsimd.indirect_dma_start(
                    out=g[:],
                    out_offset=None,
                    in_=prompt[:],
                    in_offset=bass.IndirectOffsetOnAxis(ap=idx[:, j : j + 1], axis=0),
                )
                if j == 0:
                    prev = g
                elif j == 1:
                    nc.vector.tensor_add(acc[:], prev[:], g[:])
                else:
                    nc.vector.tensor_add(acc[:], acc[:], g[:])
            if k == 1:
                nc.vector.tensor_copy(acc[:], prev[:])
            # prompted = ppg + (1/k) * sum
            nc.scalar.activation(
                acc[:], acc[:], mybir.ActivationFunctionType.Copy, bias=0.0, scale=1.0 / k
            )
            nc.vector.tensor_add(acc[:], acc[:], ppg_t[:])
            nc.gpsimd.dma_start(out=out[r0 : r0 + 128, :], in_=acc[:])

    nc.compile()
    return nc


def _numpy_fallback(ppg, keys, prompt, k):
    x = ppg[:, 0, :].astype(np.float64)
    kd = keys.astype(np.float64)
    num = x @ kd.T
    denom = np.maximum(
        np.linalg.norm(x, axis=-1, keepdims=True) * np.linalg.norm(kd, axis=-1)[None, :],
        EPS,
    )
    score_ = 1.0 - num / denom
    idx = np.argsort(score_, axis=-1, kind="stable")[:, :k]
    score = np.take_along_axis(score_, idx, axis=-1)
    m = score_.max(axis=-1, keepdims=True)
    ez = np.exp(score_ - m)
    logp = (score_ - m) - np.log(ez.sum(axis=-1, keepdims=True))
    probs = np.exp(logp)
    entropy = -(probs * logp).sum(axis=-1).mean()
    p = prompt[idx].mean(axis=1)[:, None, :]
    prompted = (ppg + p).astype(np.float32)
    return prompted, np.float32(score.mean()), np.float32(entropy)


def kernel(ppg: np.ndarray, keys: np.ndarray, prompt: np.ndarray, k) -> tuple:
    k = int(k)
    ppg = np.asarray(ppg, dtype=np.float32)
    keys = np.asarray(keys, dtype=np.float32)
    prompt = np.asarray(prompt, dtype=np.float32)
    if k < 1 or k > 8 or ppg.shape != (B_FULL, 1, L) or keys.shape != (P, L):
        return _numpy_fallback(ppg, keys, prompt, k)

    x = ppg[:, 0, :]
    xn = np.linalg.norm(x.astype(np.float64), axis=1)
    kn = np.linalg.norm(keys.astype(np.float64), axis=1)
    # denom = max(xn*kn, EPS); per-row xn scaling never changes the top-k
    # order, so fold the clamp into the key norms only (xn is ~35 for this
    # data; guard against 0 anyway).
    xn = np.maximum(xn, 1e-30)
    kn = np.maximum(kn, EPS / np.maximum(xn.min(), 1e-30))
    xTu = np.ascontiguousarray((x / xn[:, None].astype(np.float32)).T)  # [L, B]
    keysTu = np.ascontiguousarray((keys / kn[:, None].astype(np.float32)).T)  # [L, P]

    if k not in _NC_CACHE:
        _NC_CACHE[k] = _build(k)
    nc = _NC_CACHE[k]

    in_maps = []
    for c in range(N_CORES):
        b0 = c * BS
        in_maps.append(
            {
                "kx": np.ascontiguousarray(
                    np.concatenate([keysTu, xTu[:, b0 : b0 + BS]], axis=1)
                ),
                "ppg": np.ascontiguousarray(x[b0 : b0 + BS, :]),
                "prompt": prompt,
            }
        )

    global _LAST_IN_MAPS
    _LAST_IN_MAPS = in_maps

    res = run_bass_kernel_spmd(nc, in_maps, list(range(N_CORES))).results

    prompted = np.concatenate([r["out"] for r in res], axis=0)[:, None, :]
    vals = np.concatenate([r["tvals"] for r in res], axis=0)  # ns = cos-1, desc
    ents = np.concatenate([r["ent"] for r in res], axis=0)
    score_mean = np.float32(np.mean(-vals[:, :k]))
    entropy = np.float32(np.mean(ents))
    return prompted, score_mean, entropy
